# revision 25
# baseline (speedup 1.0000x reference)
"""Trainium2 Bass kernel for nn_GATModel (GATv2 on a bidirectional chain graph).

Key algebraic facts exploited (derived from the reference):
  * The reference's conv loop feeds x0 into EVERY layer, so only the LAST
    GATv2 layer (index L-1) affects the output.
  * x0 = x @ W_exp + b_exp + pe never needs materializing:
        xl = x0 @ Wl + bl = x @ (W_exp@Wl) + [(b_exp+pe[n])@Wl + bl]
  * The graph is a chain + self loops, so message passing is a 3-tap stencil
    (left / self / right) with a masked 3-way softmax per node.
  * a . leaky_relu(z) = 0.6*(a . z) + 0.4*(a . |z|)   (slope 0.2)
    and with ahat=|a| folded into the weight columns, a_h*|z_h| =
    sign(a_h)*|ztilde_h|.
  * Every per-node bias (cl~, cr~, cp, cq, cy) is a fixed function of
    n built from pe rows + constants, so they all live in one shared
    rank<=64 basis Bq over n. Stacking Bq[:, n] under x[j] in the moving
    tile lets ONE K=128 matmul produce x@W + bias(n) exactly.

Device pipeline per 500-row chunk (col-major: [h-part, row-free]):
  PE: u_b = [x;Bq] @ [Wtl_b; A_cl_b]  (2 matmuls, bias included, +2 halo
      cols), v_b likewise with Wtr/A_cr  -> PSUM f32
  ACT: evacuate u, v -> SBUF bf16
  DVE: 3 stencil adds z_{l,s,r} = shift(u)+v  ([128,2,F] bf16, 4x mode)
       + 2 abs;  GpSimd (Pool): 1 abs
  PE: t_sigma = sum_h sign(a_h)|z| via M=1 matmuls with bf16 moving
      (1 cyc/col vs 4 for f32) + P,Q,Y matmul (bias folded via Bq rows)
  evac tb (DVE/ACT alternating) -> DMA out.
Host finishes: logits = 0.6(p+q) + 0.4 t, masks, 3-way softmax, alpha-
weighted message pooling, final fc - O(B*N) work; all O(B*N*H) is on HW.

Note: the first execution of a freshly compiled NEFF intermittently hits
NRT_EXEC_UNIT_UNRECOVERABLE on this axon stack; kernel() retries.
"""

import os
import sys

sys.path.insert(0, "/opt/trn_rl_repo")

from collections import deque  # noqa: E402
from contextlib import ExitStack  # noqa: E402

import ml_dtypes  # noqa: E402
import numpy as np  # noqa: E402

import concourse.bass as bass  # noqa: E402
import concourse.tile as tile  # noqa: E402
from concourse import bacc, mybir  # noqa: E402
from concourse.bass_utils import run_bass_kernel_spmd  # noqa: E402

BF16 = mybir.dt.bfloat16
F32 = mybir.dt.float32
NPBF16 = ml_dtypes.bfloat16

B, N, IN, H, L, C = 2048, 100, 64, 256, 3, 3
NEG = 0.2
NCORES = 8
BC = B // NCORES            # 256 graphs per core
ROWS = BC * N               # 25600 rows per core
CH_ELEMS = 5
CHF = CH_ELEMS * N          # 500 rows per chunk
NFULL = BC // CH_ELEMS      # 51 full chunks
REM_ELEMS = BC - NFULL * CH_ELEMS   # 1 leftover graph
CHUNKS = [(i * CHF, CHF) for i in range(NFULL)]
if REM_ELEMS:
    CHUNKS.append((NFULL * CHF, REM_ELEMS * N))

XCOLS = 1 + ROWS + 1        # zero guard columns at 0 and ROWS+1
STRIP_W = 7 * CHF           # 3500: strips aligned to 7 chunks
NSTRIPS = (ROWS + STRIP_W - 1) // STRIP_W
SKEW = 2                    # chunks between z production and reduction
BS = CHF + 4                # 504: block stride in flat U/V/z/w layouts

LAST_RESULTS = None  # set by kernel() for test harness inspection


def _make_pe_np(n, d):
    pos = np.arange(n, dtype=np.float32)[:, None]
    div = np.exp(
        np.arange(0, d, 2, dtype=np.float32) * (-np.log(np.float32(10000.0)) / d)
    )
    pe = np.zeros((n, d), dtype=np.float32)
    pe[:, 0::2] = np.sin(pos * div)
    pe[:, 1::2] = np.cos(pos * div)
    return pe


def _install_profile_shim():
    """Best-effort: register the NTFF profile hook this container's antenv
    lacks, so BASS_TRACE=1 produces exec_time_ns instead of crashing."""
    try:
        import types

        if "antenv.axon_hooks" in sys.modules:
            return
        if "/root/.axon_site" not in sys.path:
            sys.path.insert(0, "/root/.axon_site")
        from trn_agent_boot.trn_boot import _ntff_profile_via_ctypes

        hook = _ntff_profile_via_ctypes("/opt/axon/libaxon_pjrt.so")
        mod = types.ModuleType("antenv.axon_hooks")
        mod.get_axon_ntff_profile_hook = lambda: hook
        mod.set_axon_ntff_profile_hook = lambda h: None
        sys.modules["antenv.axon_hooks"] = mod
        import antenv

        antenv.axon_hooks = mod
        import concourse.bass_utils as _bu

        _bu.upload_artifacts = lambda d: f"local://{d}"
    except Exception:
        pass


_install_profile_shim()

_PROG_CACHE = None


def _build_program():
    """Build the (shape-only) Bass program once; weights arrive via in_maps."""
    nc = bacc.Bacc(
        "TRN2",
        target_bir_lowering=False,
        debug=False,
        enable_asserts=False,
        num_devices=NCORES,
    )

    d_in = {}

    def din(name, shape, dt):
        d_in[name] = nc.dram_tensor(name, list(shape), dt, kind="ExternalInput").ap()
        return d_in[name]

    XH = din("XH", (128, XCOLS), BF16)
    SU_d = [din(f"SU{b}", (128, 128), BF16) for b in (0, 1)]
    SV_d = [din(f"SV{b}", (128, 128), BF16) for b in (0, 1)]
    SPQY_d = din("SPQY", (128, 8), BF16)
    CO_d = din("CO", (128, 2), BF16)
    outsT_dram = nc.dram_tensor("outsT", [3, ROWS], F32, kind="ExternalOutput").ap()
    outsP_dram = nc.dram_tensor("outsP", [5, ROWS], F32, kind="ExternalOutput").ap()

    with tile.TileContext(nc) as tc, ExitStack() as ctx:
        cpool = ctx.enter_context(tc.tile_pool(name="consts", bufs=1))
        spool = ctx.enter_context(tc.tile_pool(name="strips", bufs=1))
        up_pool = ctx.enter_context(
            tc.tile_pool(name="up", bufs=2, space=bass.MemorySpace.PSUM)
        )
        vp_pool = ctx.enter_context(
            tc.tile_pool(name="vp", bufs=1, space=bass.MemorySpace.PSUM)
        )
        tb_pool = ctx.enter_context(
            tc.tile_pool(name="tb", bufs=2, space=bass.MemorySpace.PSUM)
        )
        usb_pool = ctx.enter_context(tc.tile_pool(name="usb", bufs=2))
        vsb_pool = ctx.enter_context(tc.tile_pool(name="vsb", bufs=2))
        z_pool = ctx.enter_context(tc.tile_pool(name="z", bufs=SKEW + 1))
        w_pool = ctx.enter_context(tc.tile_pool(name="w", bufs=2))
        st_pool = ctx.enter_context(tc.tile_pool(name="st", bufs=2))

        def cload(name, dram_ap, shape, dt):
            t = cpool.tile(list(shape), dt, tag=f"c_{name}")
            nc.sync.dma_start(t[:], dram_ap[:])
            return t

        # strip 0 first: it gates the first chunk; consts are tiny
        strips = [None] * NSTRIPS

        def load_strip(s):
            a = STRIP_W * s
            w = min(a + STRIP_W + 2, XCOLS) - a
            t = spool.tile([128, w], BF16, tag=f"strip{s}")
            nc.sync.dma_start(t[:], XH[:, a : a + w])
            strips[s] = t

        load_strip(0)
        SU = [cload(f"su{b}", SU_d[b], (128, 128), BF16) for b in (0, 1)]
        SV = [cload(f"sv{b}", SV_d[b], (128, 128), BF16) for b in (0, 1)]
        SPQY = cload("spqy", SPQY_d, (128, 8), BF16)
        CO = cload("co", CO_d, (128, 2), BF16)
        for s in range(1, NSTRIPS):
            load_strip(s)

        def front(ci, c0, F):
            si, k = divmod(ci, 7)
            o = k * CHF
            st_t = strips[si]
            up = up_pool.tile([128, 2, 512], F32, tag="up")
            vp = vp_pool.tile([128, 2, 512], F32, tag="vp")
            for b in (0, 1):
                nc.tensor.matmul(up[:, b, 0 : F + 2], SU[b][:],
                                 st_t[:, o : o + F + 2], start=True, stop=True)
            for b in (0, 1):
                nc.tensor.matmul(vp[:, b, 0:F], SV[b][:],
                                 st_t[:, o + 1 : o + F + 1], start=True, stop=True)
            # flat SBUF layouts (block stride BS) so every DVE AP coalesces
            # to one contiguous free dim -> DVE 2x/4x fast modes
            U = usb_pool.tile([128, 2 * BS + 4], BF16, tag="usb")
            V = vsb_pool.tile([128, 2 * BS + 4], BF16, tag="vsb")
            z = z_pool.tile([128, 6 * BS], BF16, tag="z")
            W2 = 2 * BS
            Uv = U[:, 0 : 2 * BS].rearrange("p (b f) -> p b f", b=2)
            Vv = V[:, 0 : 2 * BS].rearrange("p (b f) -> p b f", b=2)
            nc.scalar.copy(Uv[:, :, 0 : F + 2], up[:, :, 0 : F + 2])
            nc.scalar.copy(Vv[:, :, 0:F], vp[:, :, 0:F])
            # all three stencil adds on DVE: GpSimd streaming SBUF in
            # parallel degrades concurrent DVE ops ~4x (port contention),
            # and routing z_s through a SWDGE accum-DMA costs more in
            # pipeline latency than the 0.7us DVE op it saves
            nc.vector.tensor_tensor(z[:, 0:W2], U[:, 0:W2],
                                    V[:, 0:W2], mybir.AluOpType.add)
            nc.vector.tensor_tensor(z[:, W2 : 2 * W2], U[:, 1 : W2 + 1],
                                    V[:, 0:W2], mybir.AluOpType.add)
            nc.vector.tensor_tensor(z[:, 2 * W2 : 3 * W2], U[:, 2 : W2 + 2],
                                    V[:, 0:W2], mybir.AluOpType.add)
            return (ci, c0, F, si, o, z)

        def back(item):
            ci, c0, F, si, o, z = item
            st_t = strips[si]
            w = w_pool.tile([128, 6 * BS], BF16, tag="w")
            I16 = mybir.dt.int16
            nc.vector.tensor_scalar(w[:, :].bitcast(I16),
                                    z[:, :].bitcast(I16), 0x7FFF, None,
                                    mybir.AluOpType.bitwise_and)
            tb = tb_pool.tile([128, 512], F32, tag="tb")
            nc.tensor.matmul(tb[96:101, 0:F], SPQY[:, 0:5],
                             st_t[:, o + 1 : o + F + 1], start=True, stop=True,
                             tile_position=(0, 96))
            W2 = 2 * BS
            for p0, sreg in ((0, 0), (32, 2 * W2), (64, W2)):
                nc.tensor.matmul(tb[p0 : p0 + 1, 0:F], CO[:, 0:1],
                                 w[:, sreg : sreg + F], start=True, stop=False)
                nc.tensor.matmul(tb[p0 : p0 + 1, 0:F], CO[:, 1:2],
                                 w[:, sreg + BS : sreg + BS + F],
                                 start=False, stop=True)
            st = st_pool.tile([128, 512], F32, tag="st")
            nc.scalar.copy(st[:, 0:F], tb[:, 0:F])
            nc.sync.dma_start(outsT_dram[0:3, c0 : c0 + F], st[0:96:32, 0:F])
            nc.sync.dma_start(outsP_dram[:, c0 : c0 + F], st[96:101, 0:F])

        pend = deque()
        for ci, (c0, F) in enumerate(CHUNKS):
            pend.append(front(ci, c0, F))
            if len(pend) > SKEW:
                back(pend.popleft())
        while pend:
            back(pend.popleft())

    nc.compile()
    return nc


def _get_program():
    global _PROG_CACHE
    if _PROG_CACHE is None:
        _PROG_CACHE = _build_program()
    return _PROG_CACHE


def kernel(x, W_exp, b_exp, W_l, b_l, W_r, b_r, att, bias, W_fc, b_fc):
    global LAST_RESULTS
    x = np.asarray(x, dtype=np.float32)
    W_exp = np.asarray(W_exp, np.float32)
    b_exp = np.asarray(b_exp, np.float32)
    W_l = np.asarray(W_l, np.float32)
    b_l = np.asarray(b_l, np.float32)
    W_r = np.asarray(W_r, np.float32)
    b_r = np.asarray(b_r, np.float32)
    att = np.asarray(att, np.float32)
    bias = np.asarray(bias, np.float32)
    W_fc = np.asarray(W_fc, np.float32)
    b_fc = np.asarray(b_fc, np.float32)

    lw = L - 1  # only the last conv layer matters
    pe = _make_pe_np(N, H)
    a = att[lw]
    s = np.where(a >= 0.0, 1.0, -1.0).astype(np.float32)
    ahat = np.abs(a)

    Wl_full = W_exp @ W_l[lw]                     # [64,256]
    Wr_full = W_exp @ W_r[lw]
    cl = (b_exp + pe) @ W_l[lw] + b_l[lw]         # [100,256]
    cr = (b_exp + pe) @ W_r[lw] + b_r[lw]

    Wtl = Wl_full * ahat[None, :]                 # ahat-folded
    Wtr = Wr_full * ahat[None, :]
    ctl = cl * ahat[None, :]
    ctr = cr * ahat[None, :]

    wp = Wl_full @ a                              # [64]
    wq = Wr_full @ a
    Wy = Wl_full @ W_fc                           # [64,3]
    cp = cl @ a                                   # [100]
    cq = cr @ a
    cy = cl @ W_fc                                # [100,3]

    # shared rank-64 basis over n for ALL per-node biases
    T = np.concatenate(
        [ctl, ctr, cp[:, None], cq[:, None], cy], axis=1
    )                                             # [100, 517]
    U_, S_, Vt_ = np.linalg.svd(T.astype(np.float64), full_matrices=False)
    Bq = U_[:, :64].T                             # [64, 100]
    A = (Bq @ T.astype(np.float64)).astype(np.float32)
    Bq = Bq.astype(np.float32)
    A_cl = A[:, 0:256]
    A_cr = A[:, 256:512]
    A_pqy = A[:, 512:517]                         # cp, cq, cy coeffs

    def bf(arr):
        return np.ascontiguousarray(arr.astype(NPBF16))

    consts = {}
    for b in (0, 1):
        sl = slice(b * 128, (b + 1) * 128)
        consts[f"SU{b}"] = bf(np.concatenate([Wtl[:, sl], A_cl[:, sl]], axis=0))
        consts[f"SV{b}"] = bf(np.concatenate([Wtr[:, sl], A_cr[:, sl]], axis=0))
    SPQY = np.zeros((128, 8), np.float32)
    SPQY[0:64, 0] = wp
    SPQY[0:64, 1] = wq
    SPQY[0:64, 2:5] = Wy
    SPQY[64:128, 0:5] = A_pqy
    consts["SPQY"] = bf(SPQY)
    CO = np.zeros((128, 2), np.float32)
    CO[:, 0] = s[0:128]
    CO[:, 1] = s[128:256]
    consts["CO"] = bf(CO)

    # per-core XH: [128, 1+ROWS+1]; rows 0:64 x^T (shifted +1 col, zero
    # guards), rows 64:128 the n-periodic basis aligned to the same cols
    xr = x.reshape(NCORES, ROWS, IN)
    n_pat = np.arange(XCOLS) % N                  # phase of col c is (c-1)%N
    basis_cols = bf(Bq[:, (n_pat - 1) % N])       # [64, XCOLS]
    in_maps = []
    for c in range(NCORES):
        XHc = np.zeros((128, XCOLS), NPBF16)
        XHc[0:64, 1 : 1 + ROWS] = bf(xr[c].T)
        XHc[64:128, :] = basis_cols
        XHc[64:128, 0] = 0
        XHc[64:128, XCOLS - 1] = 0
        m = dict(consts)
        m["XH"] = np.ascontiguousarray(XHc)
        in_maps.append(m)

    nc = _get_program()
    res = None
    last_exc = None
    for attempt in range(3):
        try:
            res = run_bass_kernel_spmd(
                nc,
                in_maps,
                core_ids=list(range(NCORES)),
            )
            break
        except Exception as e:  # transient device-unrecoverable on first NEFF run
            last_exc = e
            import time as _time

            _time.sleep(2.0)
    if res is None:
        raise last_exc
    LAST_RESULTS = res

    # ---------------- host tail ----------------
    n_of_r = np.tile(np.arange(N), BC)                        # [ROWS]

    out_all = np.empty((B, C), np.float32)
    for c in range(NCORES):
        oT = np.asarray(res.results[c]["outsT"], np.float32)  # [3, ROWS]
        oP = np.asarray(res.results[c]["outsP"], np.float32)  # [5, ROWS]
        t_l, t_r, t_s = oT[0], oT[1], oT[2]
        Pb, Qb = oP[0], oP[1]                                 # biases included
        Y = oP[2:5].T                                         # [ROWS,3]

        Pb_m1 = np.roll(Pb, 1)                                # P at source row r-1
        Pb_p1 = np.roll(Pb, -1)

        lg_l = 0.6 * (Pb_m1 + Qb) + 0.4 * t_l
        lg_r = 0.6 * (Pb_p1 + Qb) + 0.4 * t_r
        lg_s = 0.6 * (Pb + Qb) + 0.4 * t_s

        lg_l = np.where(n_of_r == 0, -np.inf, lg_l)
        lg_r = np.where(n_of_r == N - 1, -np.inf, lg_r)

        mx = np.maximum(np.maximum(lg_l, lg_r), lg_s)
        el = np.exp(lg_l - mx)
        er = np.exp(lg_r - mx)
        es = np.exp(lg_s - mx)
        den = el + er + es
        al, ar, asf = el / den, er / den, es / den

        Y_m1 = np.roll(Y, 1, axis=0)
        Y_p1 = np.roll(Y, -1, axis=0)
        msgs = al[:, None] * Y_m1 + ar[:, None] * Y_p1 + asf[:, None] * Y
        pooled = msgs.reshape(BC, N, C).sum(axis=1)
        out_all[c * BC : (c + 1) * BC] = (
            pooled + N * (bias[lw] @ W_fc)[None, :] + b_fc[None, :]
        )
    return out_all


# revision 29
# speedup vs baseline: 1.0464x; 1.0464x over previous
"""Trainium2 Bass kernel for nn_GATModel (GATv2 on a bidirectional chain graph).

Key algebraic facts exploited (derived from the reference):
  * The reference's conv loop feeds x0 into EVERY layer, so only the LAST
    GATv2 layer (index L-1) affects the output.
  * x0 = x @ W_exp + b_exp + pe never needs materializing:
        xl = x0 @ Wl + bl = x @ (W_exp@Wl) + [(b_exp+pe[n])@Wl + bl]
  * The graph is a chain + self loops, so message passing is a 3-tap stencil
    (left / self / right) with a masked 3-way softmax per node.
  * a . leaky_relu(z) = 0.6*(a . z) + 0.4*(a . |z|)   (slope 0.2)
    and with ahat=|a| folded into the weight columns, a_h*|z_h| =
    sign(a_h)*|ztilde_h|.
  * Every per-node bias (cl~, cr~, cp, cq, cy) is a fixed function of
    n built from pe rows + constants, so they all live in one shared
    rank<=64 basis Bq over n. Stacking Bq[:, n] under x[j] in the moving
    tile lets ONE K=128 matmul produce x@W + bias(n) exactly.

Device pipeline per 500-row chunk (col-major: [h-part, row-free]):
  PE: u_b = [x;Bq] @ [Wtl_b; A_cl_b]  (2 matmuls, bias included, +2 halo
      cols), v_b likewise with Wtr/A_cr  -> PSUM f32
  ACT: evacuate u, v -> SBUF bf16
  DVE: 3 stencil adds z_{l,s,r} = shift(u)+v  ([128,2,F] bf16, 4x mode)
       + 2 abs;  GpSimd (Pool): 1 abs
  PE: t_sigma = sum_h sign(a_h)|z| via M=1 matmuls with bf16 moving
      (1 cyc/col vs 4 for f32) + P,Q,Y matmul (bias folded via Bq rows)
  evac tb (DVE/ACT alternating) -> DMA out.
Host finishes: logits = 0.6(p+q) + 0.4 t, masks, 3-way softmax, alpha-
weighted message pooling, final fc - O(B*N) work; all O(B*N*H) is on HW.

Note: the first execution of a freshly compiled NEFF intermittently hits
NRT_EXEC_UNIT_UNRECOVERABLE on this axon stack; kernel() retries.
"""

import os
import sys

sys.path.insert(0, "/opt/trn_rl_repo")

from collections import deque  # noqa: E402
from contextlib import ExitStack  # noqa: E402

import ml_dtypes  # noqa: E402
import numpy as np  # noqa: E402

import concourse.bass as bass  # noqa: E402
import concourse.tile as tile  # noqa: E402
from concourse import bacc, mybir  # noqa: E402
from concourse.bass_utils import run_bass_kernel_spmd  # noqa: E402

BF16 = mybir.dt.bfloat16
F32 = mybir.dt.float32
NPBF16 = ml_dtypes.bfloat16

B, N, IN, H, L, C = 2048, 100, 64, 256, 3, 3
NEG = 0.2
NCORES = 8
BC = B // NCORES            # 256 graphs per core
ROWS = BC * N               # 25600 rows per core
CH_ELEMS = 5
CHF = CH_ELEMS * N          # 500 rows per chunk
NFULL = BC // CH_ELEMS      # 51 full chunks
REM_ELEMS = BC - NFULL * CH_ELEMS   # 1 leftover graph
CHUNKS = [(i * CHF, CHF) for i in range(NFULL)]
if REM_ELEMS:
    CHUNKS.append((NFULL * CHF, REM_ELEMS * N))

XCOLS = 1 + ROWS + 1        # zero guard columns at 0 and ROWS+1
STRIP_W = 7 * CHF           # 3500: strips aligned to 7 chunks
NSTRIPS = (ROWS + STRIP_W - 1) // STRIP_W
SKEW = 3                    # chunks between z production and reduction
BS = CHF + 4                # 504: block stride in flat U/V/z/w layouts

LAST_RESULTS = None  # set by kernel() for test harness inspection


def _make_pe_np(n, d):
    pos = np.arange(n, dtype=np.float32)[:, None]
    div = np.exp(
        np.arange(0, d, 2, dtype=np.float32) * (-np.log(np.float32(10000.0)) / d)
    )
    pe = np.zeros((n, d), dtype=np.float32)
    pe[:, 0::2] = np.sin(pos * div)
    pe[:, 1::2] = np.cos(pos * div)
    return pe


def _install_profile_shim():
    """Best-effort: register the NTFF profile hook this container's antenv
    lacks, so BASS_TRACE=1 produces exec_time_ns instead of crashing."""
    try:
        import types

        if "antenv.axon_hooks" in sys.modules:
            return
        if "/root/.axon_site" not in sys.path:
            sys.path.insert(0, "/root/.axon_site")
        from trn_agent_boot.trn_boot import _ntff_profile_via_ctypes

        hook = _ntff_profile_via_ctypes("/opt/axon/libaxon_pjrt.so")
        mod = types.ModuleType("antenv.axon_hooks")
        mod.get_axon_ntff_profile_hook = lambda: hook
        mod.set_axon_ntff_profile_hook = lambda h: None
        sys.modules["antenv.axon_hooks"] = mod
        import antenv

        antenv.axon_hooks = mod
        import concourse.bass_utils as _bu

        _bu.upload_artifacts = lambda d: f"local://{d}"
    except Exception:
        pass


_install_profile_shim()

_PROG_CACHE = None


def _build_program():
    """Build the (shape-only) Bass program once; weights arrive via in_maps."""
    nc = bacc.Bacc(
        "TRN2",
        target_bir_lowering=False,
        debug=False,
        enable_asserts=False,
        num_devices=NCORES,
    )

    d_in = {}

    def din(name, shape, dt):
        d_in[name] = nc.dram_tensor(name, list(shape), dt, kind="ExternalInput").ap()
        return d_in[name]

    XH = din("XH", (128, XCOLS), BF16)
    SU_d = [din(f"SU{b}", (128, 128), BF16) for b in (0, 1)]
    SV_d = [din(f"SV{b}", (128, 128), BF16) for b in (0, 1)]
    SPQY_d = din("SPQY", (128, 8), BF16)
    CO_d = din("CO", (128, 2), BF16)
    outsT_dram = nc.dram_tensor("outsT", [3, ROWS], F32, kind="ExternalOutput").ap()
    outsP_dram = nc.dram_tensor("outsP", [5, ROWS], F32, kind="ExternalOutput").ap()

    with tile.TileContext(nc) as tc, ExitStack() as ctx:
        cpool = ctx.enter_context(tc.tile_pool(name="consts", bufs=1))
        spool = ctx.enter_context(tc.tile_pool(name="strips", bufs=1))
        up_pool = ctx.enter_context(
            tc.tile_pool(name="up", bufs=2, space=bass.MemorySpace.PSUM)
        )
        vp_pool = ctx.enter_context(
            tc.tile_pool(name="vp", bufs=1, space=bass.MemorySpace.PSUM)
        )
        tb_pool = ctx.enter_context(
            tc.tile_pool(name="tb", bufs=2, space=bass.MemorySpace.PSUM)
        )
        usb_pool = ctx.enter_context(tc.tile_pool(name="usb", bufs=2))
        vsb_pool = ctx.enter_context(tc.tile_pool(name="vsb", bufs=2))
        z_pool = ctx.enter_context(tc.tile_pool(name="z", bufs=SKEW + 1))
        w_pool = ctx.enter_context(tc.tile_pool(name="w", bufs=2))
        st_pool = ctx.enter_context(tc.tile_pool(name="st", bufs=2))

        def cload(name, dram_ap, shape, dt):
            t = cpool.tile(list(shape), dt, tag=f"c_{name}")
            nc.sync.dma_start(t[:], dram_ap[:])
            return t

        # tiny head tile first (gates chunks 0-1), then consts, then strips
        strips = [None] * NSTRIPS
        head = spool.tile([128, 2 * CHF + 6], BF16, tag="head")
        nc.sync.dma_start(head[:], XH[:, 0 : 2 * CHF + 6])

        def load_strip(s):
            a = STRIP_W * s
            w = min(a + STRIP_W + 2, XCOLS) - a
            t = spool.tile([128, w], BF16, tag=f"strip{s}")
            nc.sync.dma_start(t[:], XH[:, a : a + w])
            strips[s] = t

        SU = [cload(f"su{b}", SU_d[b], (128, 128), BF16) for b in (0, 1)]
        SV = [cload(f"sv{b}", SV_d[b], (128, 128), BF16) for b in (0, 1)]
        SPQY = cload("spqy", SPQY_d, (128, 8), BF16)
        CO = cload("co", CO_d, (128, 2), BF16)
        for s in range(NSTRIPS):
            load_strip(s)

        def moving_for(ci):
            if ci < 2:
                return head, ci * CHF
            si, k = divmod(ci, 7)
            return strips[si], k * CHF

        def front(ci, c0, F):
            st_t, o = moving_for(ci)
            up = up_pool.tile([128, 2, 512], F32, tag="up")
            vp = vp_pool.tile([128, 2, 512], F32, tag="vp")
            for b in (0, 1):
                nc.tensor.matmul(up[:, b, 0 : F + 2], SU[b][:],
                                 st_t[:, o : o + F + 2], start=True, stop=True)
            for b in (0, 1):
                nc.tensor.matmul(vp[:, b, 0:F], SV[b][:],
                                 st_t[:, o + 1 : o + F + 1], start=True, stop=True)
            # flat SBUF layouts (block stride BS) so every DVE AP coalesces
            # to one contiguous free dim -> DVE 2x/4x fast modes
            U = usb_pool.tile([128, 2 * BS + 4], BF16, tag="usb")
            V = vsb_pool.tile([128, 2 * BS + 4], BF16, tag="vsb")
            z = z_pool.tile([128, 6 * BS], BF16, tag="z")
            W2 = 2 * BS
            Uv = U[:, 0 : 2 * BS].rearrange("p (b f) -> p b f", b=2)
            Vv = V[:, 0 : 2 * BS].rearrange("p (b f) -> p b f", b=2)
            nc.scalar.copy(Uv[:, :, 0 : F + 2], up[:, :, 0 : F + 2])
            nc.scalar.copy(Vv[:, :, 0:F], vp[:, :, 0:F])
            # all three stencil adds on DVE: GpSimd streaming SBUF in
            # parallel degrades concurrent DVE ops ~4x (port contention),
            # and routing z_s through a SWDGE accum-DMA costs more in
            # pipeline latency than the 0.7us DVE op it saves
            nc.vector.tensor_tensor(z[:, 0:W2], U[:, 0:W2],
                                    V[:, 0:W2], mybir.AluOpType.add)
            nc.vector.tensor_tensor(z[:, W2 : 2 * W2], U[:, 1 : W2 + 1],
                                    V[:, 0:W2], mybir.AluOpType.add)
            nc.vector.tensor_tensor(z[:, 2 * W2 : 3 * W2], U[:, 2 : W2 + 2],
                                    V[:, 0:W2], mybir.AluOpType.add)
            return (ci, c0, F, st_t, o, z)

        def back(item):
            ci, c0, F, st_t, o, z = item
            w = w_pool.tile([128, 6 * BS], BF16, tag="w")
            I16 = mybir.dt.int16
            nc.vector.tensor_scalar(w[:, :].bitcast(I16),
                                    z[:, :].bitcast(I16), 0x7FFF, None,
                                    mybir.AluOpType.bitwise_and)
            tb = tb_pool.tile([128, 512], F32, tag="tb")
            nc.tensor.matmul(tb[96:101, 0:F], SPQY[:, 0:5],
                             st_t[:, o + 1 : o + F + 1], start=True, stop=True,
                             tile_position=(0, 96))
            W2 = 2 * BS
            for p0, sreg in ((0, 0), (32, 2 * W2), (64, W2)):
                nc.tensor.matmul(tb[p0 : p0 + 1, 0:F], CO[:, 0:1],
                                 w[:, sreg : sreg + F], start=True, stop=False)
                nc.tensor.matmul(tb[p0 : p0 + 1, 0:F], CO[:, 1:2],
                                 w[:, sreg + BS : sreg + BS + F],
                                 start=False, stop=True)
            st = st_pool.tile([128, 512], F32, tag="st")
            nc.scalar.copy(st[:, 0:F], tb[:, 0:F])
            nc.sync.dma_start(outsT_dram[0:3, c0 : c0 + F], st[0:96:32, 0:F])
            nc.sync.dma_start(outsP_dram[:, c0 : c0 + F], st[96:101, 0:F])

        pend = deque()
        for ci, (c0, F) in enumerate(CHUNKS):
            pend.append(front(ci, c0, F))
            if len(pend) > SKEW:
                back(pend.popleft())
        while pend:
            back(pend.popleft())

    nc.compile()
    return nc


def _get_program():
    global _PROG_CACHE
    if _PROG_CACHE is None:
        _PROG_CACHE = _build_program()
    return _PROG_CACHE


def kernel(x, W_exp, b_exp, W_l, b_l, W_r, b_r, att, bias, W_fc, b_fc):
    global LAST_RESULTS
    x = np.asarray(x, dtype=np.float32)
    W_exp = np.asarray(W_exp, np.float32)
    b_exp = np.asarray(b_exp, np.float32)
    W_l = np.asarray(W_l, np.float32)
    b_l = np.asarray(b_l, np.float32)
    W_r = np.asarray(W_r, np.float32)
    b_r = np.asarray(b_r, np.float32)
    att = np.asarray(att, np.float32)
    bias = np.asarray(bias, np.float32)
    W_fc = np.asarray(W_fc, np.float32)
    b_fc = np.asarray(b_fc, np.float32)

    lw = L - 1  # only the last conv layer matters
    pe = _make_pe_np(N, H)
    a = att[lw]
    s = np.where(a >= 0.0, 1.0, -1.0).astype(np.float32)
    ahat = np.abs(a)

    Wl_full = W_exp @ W_l[lw]                     # [64,256]
    Wr_full = W_exp @ W_r[lw]
    cl = (b_exp + pe) @ W_l[lw] + b_l[lw]         # [100,256]
    cr = (b_exp + pe) @ W_r[lw] + b_r[lw]

    Wtl = Wl_full * ahat[None, :]                 # ahat-folded
    Wtr = Wr_full * ahat[None, :]
    ctl = cl * ahat[None, :]
    ctr = cr * ahat[None, :]

    wp = Wl_full @ a                              # [64]
    wq = Wr_full @ a
    Wy = Wl_full @ W_fc                           # [64,3]
    cp = cl @ a                                   # [100]
    cq = cr @ a
    cy = cl @ W_fc                                # [100,3]

    # shared rank-64 basis over n for ALL per-node biases
    T = np.concatenate(
        [ctl, ctr, cp[:, None], cq[:, None], cy], axis=1
    )                                             # [100, 517]
    U_, S_, Vt_ = np.linalg.svd(T.astype(np.float64), full_matrices=False)
    Bq = U_[:, :64].T                             # [64, 100]
    A = (Bq @ T.astype(np.float64)).astype(np.float32)
    Bq = Bq.astype(np.float32)
    A_cl = A[:, 0:256]
    A_cr = A[:, 256:512]
    A_pqy = A[:, 512:517]                         # cp, cq, cy coeffs

    def bf(arr):
        return np.ascontiguousarray(arr.astype(NPBF16))

    consts = {}
    for b in (0, 1):
        sl = slice(b * 128, (b + 1) * 128)
        consts[f"SU{b}"] = bf(np.concatenate([Wtl[:, sl], A_cl[:, sl]], axis=0))
        consts[f"SV{b}"] = bf(np.concatenate([Wtr[:, sl], A_cr[:, sl]], axis=0))
    SPQY = np.zeros((128, 8), np.float32)
    SPQY[0:64, 0] = wp
    SPQY[0:64, 1] = wq
    SPQY[0:64, 2:5] = Wy
    SPQY[64:128, 0:5] = A_pqy
    consts["SPQY"] = bf(SPQY)
    CO = np.zeros((128, 2), np.float32)
    CO[:, 0] = s[0:128]
    CO[:, 1] = s[128:256]
    consts["CO"] = bf(CO)

    # per-core XH: [128, 1+ROWS+1]; rows 0:64 x^T (shifted +1 col, zero
    # guards), rows 64:128 the n-periodic basis aligned to the same cols
    xr = x.reshape(NCORES, ROWS, IN)
    n_pat = np.arange(XCOLS) % N                  # phase of col c is (c-1)%N
    basis_cols = bf(Bq[:, (n_pat - 1) % N])       # [64, XCOLS]
    in_maps = []
    for c in range(NCORES):
        XHc = np.zeros((128, XCOLS), NPBF16)
        XHc[0:64, 1 : 1 + ROWS] = bf(xr[c].T)
        XHc[64:128, :] = basis_cols
        XHc[64:128, 0] = 0
        XHc[64:128, XCOLS - 1] = 0
        m = dict(consts)
        m["XH"] = np.ascontiguousarray(XHc)
        in_maps.append(m)

    nc = _get_program()
    res = None
    last_exc = None
    for attempt in range(3):
        try:
            res = run_bass_kernel_spmd(
                nc,
                in_maps,
                core_ids=list(range(NCORES)),
            )
            break
        except Exception as e:  # transient device-unrecoverable on first NEFF run
            last_exc = e
            import time as _time

            _time.sleep(2.0)
    if res is None:
        raise last_exc
    LAST_RESULTS = res

    # ---------------- host tail ----------------
    n_of_r = np.tile(np.arange(N), BC)                        # [ROWS]

    out_all = np.empty((B, C), np.float32)
    for c in range(NCORES):
        oT = np.asarray(res.results[c]["outsT"], np.float32)  # [3, ROWS]
        oP = np.asarray(res.results[c]["outsP"], np.float32)  # [5, ROWS]
        t_l, t_r, t_s = oT[0], oT[1], oT[2]
        Pb, Qb = oP[0], oP[1]                                 # biases included
        Y = oP[2:5].T                                         # [ROWS,3]

        Pb_m1 = np.roll(Pb, 1)                                # P at source row r-1
        Pb_p1 = np.roll(Pb, -1)

        lg_l = 0.6 * (Pb_m1 + Qb) + 0.4 * t_l
        lg_r = 0.6 * (Pb_p1 + Qb) + 0.4 * t_r
        lg_s = 0.6 * (Pb + Qb) + 0.4 * t_s

        lg_l = np.where(n_of_r == 0, -np.inf, lg_l)
        lg_r = np.where(n_of_r == N - 1, -np.inf, lg_r)

        mx = np.maximum(np.maximum(lg_l, lg_r), lg_s)
        el = np.exp(lg_l - mx)
        er = np.exp(lg_r - mx)
        es = np.exp(lg_s - mx)
        den = el + er + es
        al, ar, asf = el / den, er / den, es / den

        Y_m1 = np.roll(Y, 1, axis=0)
        Y_p1 = np.roll(Y, -1, axis=0)
        msgs = al[:, None] * Y_m1 + ar[:, None] * Y_p1 + asf[:, None] * Y
        pooled = msgs.reshape(BC, N, C).sum(axis=1)
        out_all[c * BC : (c + 1) * BC] = (
            pooled + N * (bias[lw] @ W_fc)[None, :] + b_fc[None, :]
        )
    return out_all


# revision 30
# speedup vs baseline: 1.0521x; 1.0054x over previous
"""Trainium2 Bass kernel for nn_GATModel (GATv2 on a bidirectional chain graph).

Key algebraic facts exploited (derived from the reference):
  * The reference's conv loop feeds x0 into EVERY layer, so only the LAST
    GATv2 layer (index L-1) affects the output.
  * x0 = x @ W_exp + b_exp + pe never needs materializing:
        xl = x0 @ Wl + bl = x @ (W_exp@Wl) + [(b_exp+pe[n])@Wl + bl]
  * The graph is a chain + self loops, so message passing is a 3-tap stencil
    (left / self / right) with a masked 3-way softmax per node.
  * a . leaky_relu(z) = 0.6*(a . z) + 0.4*(a . |z|)   (slope 0.2)
    and with ahat=|a| folded into the weight columns, a_h*|z_h| =
    sign(a_h)*|ztilde_h|.
  * Every per-node bias (cl~, cr~, cp, cq, cy) is a fixed function of
    n built from pe rows + constants, so they all live in one shared
    rank<=64 basis Bq over n. Stacking Bq[:, n] under x[j] in the moving
    tile lets ONE K=128 matmul produce x@W + bias(n) exactly.

Device pipeline per 500-row chunk (col-major: [h-part, row-free]):
  PE: u_b = [x;Bq] @ [Wtl_b; A_cl_b]  (2 matmuls, bias included, +2 halo
      cols), v_b likewise with Wtr/A_cr  -> PSUM f32
  ACT: evacuate u, v -> SBUF bf16
  DVE: 3 stencil adds z_{l,s,r} = shift(u)+v  ([128,2,F] bf16, 4x mode)
       + 2 abs;  GpSimd (Pool): 1 abs
  PE: t_sigma = sum_h sign(a_h)|z| via M=1 matmuls with bf16 moving
      (1 cyc/col vs 4 for f32) + P,Q,Y matmul (bias folded via Bq rows)
  evac tb (DVE/ACT alternating) -> DMA out.
Host finishes: logits = 0.6(p+q) + 0.4 t, masks, 3-way softmax, alpha-
weighted message pooling, final fc - O(B*N) work; all O(B*N*H) is on HW.

Note: the first execution of a freshly compiled NEFF intermittently hits
NRT_EXEC_UNIT_UNRECOVERABLE on this axon stack; kernel() retries.
"""

import os
import sys

sys.path.insert(0, "/opt/trn_rl_repo")

from collections import deque  # noqa: E402
from contextlib import ExitStack  # noqa: E402

import ml_dtypes  # noqa: E402
import numpy as np  # noqa: E402

import concourse.bass as bass  # noqa: E402
import concourse.tile as tile  # noqa: E402
from concourse import bacc, mybir  # noqa: E402
from concourse.bass_utils import run_bass_kernel_spmd  # noqa: E402

BF16 = mybir.dt.bfloat16
F32 = mybir.dt.float32
NPBF16 = ml_dtypes.bfloat16

B, N, IN, H, L, C = 2048, 100, 64, 256, 3, 3
NEG = 0.2
NCORES = 8
BC = B // NCORES            # 256 graphs per core
ROWS = BC * N               # 25600 rows per core
CH_ELEMS = 5
CHF = CH_ELEMS * N          # 500 rows per chunk
NFULL = BC // CH_ELEMS      # 51 full chunks
REM_ELEMS = BC - NFULL * CH_ELEMS   # 1 leftover graph
CHUNKS = [(i * CHF, CHF) for i in range(NFULL)]
if REM_ELEMS:
    CHUNKS.append((NFULL * CHF, REM_ELEMS * N))

XCOLS = 1 + ROWS + 1        # zero guard columns at 0 and ROWS+1
STRIP_W = 7 * CHF           # 3500: strips aligned to 7 chunks
NSTRIPS = (ROWS + STRIP_W - 1) // STRIP_W
SKEW = 4                    # chunks between z production and reduction
BS = CHF + 4                # 504: block stride in flat U/V/z/w layouts

LAST_RESULTS = None  # set by kernel() for test harness inspection


def _make_pe_np(n, d):
    pos = np.arange(n, dtype=np.float32)[:, None]
    div = np.exp(
        np.arange(0, d, 2, dtype=np.float32) * (-np.log(np.float32(10000.0)) / d)
    )
    pe = np.zeros((n, d), dtype=np.float32)
    pe[:, 0::2] = np.sin(pos * div)
    pe[:, 1::2] = np.cos(pos * div)
    return pe


def _install_profile_shim():
    """Best-effort: register the NTFF profile hook this container's antenv
    lacks, so BASS_TRACE=1 produces exec_time_ns instead of crashing."""
    try:
        import types

        if "antenv.axon_hooks" in sys.modules:
            return
        if "/root/.axon_site" not in sys.path:
            sys.path.insert(0, "/root/.axon_site")
        from trn_agent_boot.trn_boot import _ntff_profile_via_ctypes

        hook = _ntff_profile_via_ctypes("/opt/axon/libaxon_pjrt.so")
        mod = types.ModuleType("antenv.axon_hooks")
        mod.get_axon_ntff_profile_hook = lambda: hook
        mod.set_axon_ntff_profile_hook = lambda h: None
        sys.modules["antenv.axon_hooks"] = mod
        import antenv

        antenv.axon_hooks = mod
        import concourse.bass_utils as _bu

        _bu.upload_artifacts = lambda d: f"local://{d}"
    except Exception:
        pass


_install_profile_shim()

_PROG_CACHE = None


def _build_program():
    """Build the (shape-only) Bass program once; weights arrive via in_maps."""
    nc = bacc.Bacc(
        "TRN2",
        target_bir_lowering=False,
        debug=False,
        enable_asserts=False,
        num_devices=NCORES,
    )

    d_in = {}

    def din(name, shape, dt):
        d_in[name] = nc.dram_tensor(name, list(shape), dt, kind="ExternalInput").ap()
        return d_in[name]

    XH = din("XH", (128, XCOLS), BF16)
    SU_d = [din(f"SU{b}", (128, 128), BF16) for b in (0, 1)]
    SV_d = [din(f"SV{b}", (128, 128), BF16) for b in (0, 1)]
    SPQY_d = din("SPQY", (128, 8), BF16)
    CO_d = din("CO", (128, 2), BF16)
    outsT_dram = nc.dram_tensor("outsT", [3, ROWS], F32, kind="ExternalOutput").ap()
    outsP_dram = nc.dram_tensor("outsP", [5, ROWS], F32, kind="ExternalOutput").ap()

    with tile.TileContext(nc) as tc, ExitStack() as ctx:
        cpool = ctx.enter_context(tc.tile_pool(name="consts", bufs=1))
        spool = ctx.enter_context(tc.tile_pool(name="strips", bufs=1))
        up_pool = ctx.enter_context(
            tc.tile_pool(name="up", bufs=2, space=bass.MemorySpace.PSUM)
        )
        vp_pool = ctx.enter_context(
            tc.tile_pool(name="vp", bufs=1, space=bass.MemorySpace.PSUM)
        )
        tb_pool = ctx.enter_context(
            tc.tile_pool(name="tb", bufs=2, space=bass.MemorySpace.PSUM)
        )
        usb_pool = ctx.enter_context(tc.tile_pool(name="usb", bufs=2))
        vsb_pool = ctx.enter_context(tc.tile_pool(name="vsb", bufs=2))
        z_pool = ctx.enter_context(tc.tile_pool(name="z", bufs=SKEW + 1))
        w_pool = ctx.enter_context(tc.tile_pool(name="w", bufs=2))
        st_pool = ctx.enter_context(tc.tile_pool(name="st", bufs=2))

        def cload(name, dram_ap, shape, dt):
            t = cpool.tile(list(shape), dt, tag=f"c_{name}")
            nc.sync.dma_start(t[:], dram_ap[:])
            return t

        # tiny head tile first (gates chunks 0-1), then consts, then strips
        strips = [None] * NSTRIPS
        head = spool.tile([128, 2 * CHF + 6], BF16, tag="head")
        nc.sync.dma_start(head[:], XH[:, 0 : 2 * CHF + 6])

        def load_strip(s):
            a = STRIP_W * s
            w = min(a + STRIP_W + 2, XCOLS) - a
            t = spool.tile([128, w], BF16, tag=f"strip{s}")
            nc.sync.dma_start(t[:], XH[:, a : a + w])
            strips[s] = t

        SU = [cload(f"su{b}", SU_d[b], (128, 128), BF16) for b in (0, 1)]
        SV = [cload(f"sv{b}", SV_d[b], (128, 128), BF16) for b in (0, 1)]
        SPQY = cload("spqy", SPQY_d, (128, 8), BF16)
        CO = cload("co", CO_d, (128, 2), BF16)
        for s in range(NSTRIPS):
            load_strip(s)

        def moving_for(ci):
            if ci < 2:
                return head, ci * CHF
            si, k = divmod(ci, 7)
            return strips[si], k * CHF

        def front(ci, c0, F):
            st_t, o = moving_for(ci)
            up = up_pool.tile([128, 2, 512], F32, tag="up")
            vp = vp_pool.tile([128, 2, 512], F32, tag="vp")
            for b in (0, 1):
                nc.tensor.matmul(up[:, b, 0 : F + 2], SU[b][:],
                                 st_t[:, o : o + F + 2], start=True, stop=True)
            for b in (0, 1):
                nc.tensor.matmul(vp[:, b, 0:F], SV[b][:],
                                 st_t[:, o + 1 : o + F + 1], start=True, stop=True)
            # flat SBUF layouts (block stride BS) so every DVE AP coalesces
            # to one contiguous free dim -> DVE 2x/4x fast modes
            U = usb_pool.tile([128, 2 * BS + 4], BF16, tag="usb")
            V = vsb_pool.tile([128, 2 * BS + 4], BF16, tag="vsb")
            z = z_pool.tile([128, 6 * BS], BF16, tag="z")
            W2 = 2 * BS
            Uv = U[:, 0 : 2 * BS].rearrange("p (b f) -> p b f", b=2)
            Vv = V[:, 0 : 2 * BS].rearrange("p (b f) -> p b f", b=2)
            nc.scalar.copy(Uv[:, :, 0 : F + 2], up[:, :, 0 : F + 2])
            nc.scalar.copy(Vv[:, :, 0:F], vp[:, :, 0:F])
            # all three stencil adds on DVE: GpSimd streaming SBUF in
            # parallel degrades concurrent DVE ops ~4x (port contention),
            # and routing z_s through a SWDGE accum-DMA costs more in
            # pipeline latency than the 0.7us DVE op it saves
            nc.vector.tensor_tensor(z[:, 0:W2], U[:, 0:W2],
                                    V[:, 0:W2], mybir.AluOpType.add)
            nc.vector.tensor_tensor(z[:, W2 : 2 * W2], U[:, 1 : W2 + 1],
                                    V[:, 0:W2], mybir.AluOpType.add)
            nc.vector.tensor_tensor(z[:, 2 * W2 : 3 * W2], U[:, 2 : W2 + 2],
                                    V[:, 0:W2], mybir.AluOpType.add)
            return (ci, c0, F, st_t, o, z)

        def back(item):
            ci, c0, F, st_t, o, z = item
            w = w_pool.tile([128, 6 * BS], BF16, tag="w")
            I16 = mybir.dt.int16
            nc.vector.tensor_scalar(w[:, :].bitcast(I16),
                                    z[:, :].bitcast(I16), 0x7FFF, None,
                                    mybir.AluOpType.bitwise_and)
            tb = tb_pool.tile([128, 512], F32, tag="tb")
            nc.tensor.matmul(tb[96:101, 0:F], SPQY[:, 0:5],
                             st_t[:, o + 1 : o + F + 1], start=True, stop=True,
                             tile_position=(0, 96))
            W2 = 2 * BS
            for p0, sreg in ((0, 0), (32, 2 * W2), (64, W2)):
                nc.tensor.matmul(tb[p0 : p0 + 1, 0:F], CO[:, 0:1],
                                 w[:, sreg : sreg + F], start=True, stop=False)
                nc.tensor.matmul(tb[p0 : p0 + 1, 0:F], CO[:, 1:2],
                                 w[:, sreg + BS : sreg + BS + F],
                                 start=False, stop=True)
            st = st_pool.tile([128, 512], F32, tag="st")
            nc.scalar.copy(st[:, 0:F], tb[:, 0:F])
            nc.sync.dma_start(outsT_dram[0:3, c0 : c0 + F], st[0:96:32, 0:F])
            nc.sync.dma_start(outsP_dram[:, c0 : c0 + F], st[96:101, 0:F])

        pend = deque()
        for ci, (c0, F) in enumerate(CHUNKS):
            pend.append(front(ci, c0, F))
            if len(pend) > SKEW:
                back(pend.popleft())
        while pend:
            back(pend.popleft())

    nc.compile()
    return nc


def _get_program():
    global _PROG_CACHE
    if _PROG_CACHE is None:
        _PROG_CACHE = _build_program()
    return _PROG_CACHE


def kernel(x, W_exp, b_exp, W_l, b_l, W_r, b_r, att, bias, W_fc, b_fc):
    global LAST_RESULTS
    x = np.asarray(x, dtype=np.float32)
    W_exp = np.asarray(W_exp, np.float32)
    b_exp = np.asarray(b_exp, np.float32)
    W_l = np.asarray(W_l, np.float32)
    b_l = np.asarray(b_l, np.float32)
    W_r = np.asarray(W_r, np.float32)
    b_r = np.asarray(b_r, np.float32)
    att = np.asarray(att, np.float32)
    bias = np.asarray(bias, np.float32)
    W_fc = np.asarray(W_fc, np.float32)
    b_fc = np.asarray(b_fc, np.float32)

    lw = L - 1  # only the last conv layer matters
    pe = _make_pe_np(N, H)
    a = att[lw]
    s = np.where(a >= 0.0, 1.0, -1.0).astype(np.float32)
    ahat = np.abs(a)

    Wl_full = W_exp @ W_l[lw]                     # [64,256]
    Wr_full = W_exp @ W_r[lw]
    cl = (b_exp + pe) @ W_l[lw] + b_l[lw]         # [100,256]
    cr = (b_exp + pe) @ W_r[lw] + b_r[lw]

    Wtl = Wl_full * ahat[None, :]                 # ahat-folded
    Wtr = Wr_full * ahat[None, :]
    ctl = cl * ahat[None, :]
    ctr = cr * ahat[None, :]

    wp = Wl_full @ a                              # [64]
    wq = Wr_full @ a
    Wy = Wl_full @ W_fc                           # [64,3]
    cp = cl @ a                                   # [100]
    cq = cr @ a
    cy = cl @ W_fc                                # [100,3]

    # shared rank-64 basis over n for ALL per-node biases
    T = np.concatenate(
        [ctl, ctr, cp[:, None], cq[:, None], cy], axis=1
    )                                             # [100, 517]
    U_, S_, Vt_ = np.linalg.svd(T.astype(np.float64), full_matrices=False)
    Bq = U_[:, :64].T                             # [64, 100]
    A = (Bq @ T.astype(np.float64)).astype(np.float32)
    Bq = Bq.astype(np.float32)
    A_cl = A[:, 0:256]
    A_cr = A[:, 256:512]
    A_pqy = A[:, 512:517]                         # cp, cq, cy coeffs

    def bf(arr):
        return np.ascontiguousarray(arr.astype(NPBF16))

    consts = {}
    for b in (0, 1):
        sl = slice(b * 128, (b + 1) * 128)
        consts[f"SU{b}"] = bf(np.concatenate([Wtl[:, sl], A_cl[:, sl]], axis=0))
        consts[f"SV{b}"] = bf(np.concatenate([Wtr[:, sl], A_cr[:, sl]], axis=0))
    SPQY = np.zeros((128, 8), np.float32)
    SPQY[0:64, 0] = wp
    SPQY[0:64, 1] = wq
    SPQY[0:64, 2:5] = Wy
    SPQY[64:128, 0:5] = A_pqy
    consts["SPQY"] = bf(SPQY)
    CO = np.zeros((128, 2), np.float32)
    CO[:, 0] = s[0:128]
    CO[:, 1] = s[128:256]
    consts["CO"] = bf(CO)

    # per-core XH: [128, 1+ROWS+1]; rows 0:64 x^T (shifted +1 col, zero
    # guards), rows 64:128 the n-periodic basis aligned to the same cols
    xr = x.reshape(NCORES, ROWS, IN)
    n_pat = np.arange(XCOLS) % N                  # phase of col c is (c-1)%N
    basis_cols = bf(Bq[:, (n_pat - 1) % N])       # [64, XCOLS]
    in_maps = []
    for c in range(NCORES):
        XHc = np.zeros((128, XCOLS), NPBF16)
        XHc[0:64, 1 : 1 + ROWS] = bf(xr[c].T)
        XHc[64:128, :] = basis_cols
        XHc[64:128, 0] = 0
        XHc[64:128, XCOLS - 1] = 0
        m = dict(consts)
        m["XH"] = np.ascontiguousarray(XHc)
        in_maps.append(m)

    nc = _get_program()
    res = None
    last_exc = None
    for attempt in range(3):
        try:
            res = run_bass_kernel_spmd(
                nc,
                in_maps,
                core_ids=list(range(NCORES)),
            )
            break
        except Exception as e:  # transient device-unrecoverable on first NEFF run
            last_exc = e
            import time as _time

            _time.sleep(2.0)
    if res is None:
        raise last_exc
    LAST_RESULTS = res

    # ---------------- host tail ----------------
    n_of_r = np.tile(np.arange(N), BC)                        # [ROWS]

    out_all = np.empty((B, C), np.float32)
    for c in range(NCORES):
        oT = np.asarray(res.results[c]["outsT"], np.float32)  # [3, ROWS]
        oP = np.asarray(res.results[c]["outsP"], np.float32)  # [5, ROWS]
        t_l, t_r, t_s = oT[0], oT[1], oT[2]
        Pb, Qb = oP[0], oP[1]                                 # biases included
        Y = oP[2:5].T                                         # [ROWS,3]

        Pb_m1 = np.roll(Pb, 1)                                # P at source row r-1
        Pb_p1 = np.roll(Pb, -1)

        lg_l = 0.6 * (Pb_m1 + Qb) + 0.4 * t_l
        lg_r = 0.6 * (Pb_p1 + Qb) + 0.4 * t_r
        lg_s = 0.6 * (Pb + Qb) + 0.4 * t_s

        lg_l = np.where(n_of_r == 0, -np.inf, lg_l)
        lg_r = np.where(n_of_r == N - 1, -np.inf, lg_r)

        mx = np.maximum(np.maximum(lg_l, lg_r), lg_s)
        el = np.exp(lg_l - mx)
        er = np.exp(lg_r - mx)
        es = np.exp(lg_s - mx)
        den = el + er + es
        al, ar, asf = el / den, er / den, es / den

        Y_m1 = np.roll(Y, 1, axis=0)
        Y_p1 = np.roll(Y, -1, axis=0)
        msgs = al[:, None] * Y_m1 + ar[:, None] * Y_p1 + asf[:, None] * Y
        pooled = msgs.reshape(BC, N, C).sum(axis=1)
        out_all[c * BC : (c + 1) * BC] = (
            pooled + N * (bias[lw] @ W_fc)[None, :] + b_fc[None, :]
        )
    return out_all


# revision 31
# speedup vs baseline: 1.0551x; 1.0028x over previous
"""Trainium2 Bass kernel for nn_GATModel (GATv2 on a bidirectional chain graph).

Key algebraic facts exploited (derived from the reference):
  * The reference's conv loop feeds x0 into EVERY layer, so only the LAST
    GATv2 layer (index L-1) affects the output.
  * x0 = x @ W_exp + b_exp + pe never needs materializing:
        xl = x0 @ Wl + bl = x @ (W_exp@Wl) + [(b_exp+pe[n])@Wl + bl]
  * The graph is a chain + self loops, so message passing is a 3-tap stencil
    (left / self / right) with a masked 3-way softmax per node.
  * a . leaky_relu(z) = 0.6*(a . z) + 0.4*(a . |z|)   (slope 0.2)
    and with ahat=|a| folded into the weight columns, a_h*|z_h| =
    sign(a_h)*|ztilde_h|.
  * Every per-node bias (cl~, cr~, cp, cq, cy) is a fixed function of
    n built from pe rows + constants, so they all live in one shared
    rank<=64 basis Bq over n. Stacking Bq[:, n] under x[j] in the moving
    tile lets ONE K=128 matmul produce x@W + bias(n) exactly.

Device pipeline per 500-row chunk (col-major: [h-part, row-free]):
  PE: u_b = [x;Bq] @ [Wtl_b; A_cl_b]  (2 matmuls, bias included, +2 halo
      cols), v_b likewise with Wtr/A_cr  -> PSUM f32
  ACT: evacuate u, v -> SBUF bf16
  DVE: 3 stencil adds z_{l,s,r} = shift(u)+v  ([128,2,F] bf16, 4x mode)
       + 2 abs;  GpSimd (Pool): 1 abs
  PE: t_sigma = sum_h sign(a_h)|z| via M=1 matmuls with bf16 moving
      (1 cyc/col vs 4 for f32) + P,Q,Y matmul (bias folded via Bq rows)
  evac tb (DVE/ACT alternating) -> DMA out.
Host finishes: logits = 0.6(p+q) + 0.4 t, masks, 3-way softmax, alpha-
weighted message pooling, final fc - O(B*N) work; all O(B*N*H) is on HW.

Note: the first execution of a freshly compiled NEFF intermittently hits
NRT_EXEC_UNIT_UNRECOVERABLE on this axon stack; kernel() retries.
"""

import os
import sys

sys.path.insert(0, "/opt/trn_rl_repo")

from collections import deque  # noqa: E402
from contextlib import ExitStack  # noqa: E402

import ml_dtypes  # noqa: E402
import numpy as np  # noqa: E402

import concourse.bass as bass  # noqa: E402
import concourse.tile as tile  # noqa: E402
from concourse import bacc, mybir  # noqa: E402
from concourse.bass_utils import run_bass_kernel_spmd  # noqa: E402

BF16 = mybir.dt.bfloat16
F32 = mybir.dt.float32
NPBF16 = ml_dtypes.bfloat16

B, N, IN, H, L, C = 2048, 100, 64, 256, 3, 3
NEG = 0.2
NCORES = 8
BC = B // NCORES            # 256 graphs per core
ROWS = BC * N               # 25600 rows per core
CH_ELEMS = 5
CHF = CH_ELEMS * N          # 500 rows per chunk
NFULL = BC // CH_ELEMS      # 51 full chunks
REM_ELEMS = BC - NFULL * CH_ELEMS   # 1 leftover graph
CHUNKS = [(i * CHF, CHF) for i in range(NFULL)]
if REM_ELEMS:
    CHUNKS.append((NFULL * CHF, REM_ELEMS * N))

XCOLS = 1 + ROWS + 1        # zero guard columns at 0 and ROWS+1
STRIP_W = 7 * CHF           # 3500: strips aligned to 7 chunks
NSTRIPS = (ROWS + STRIP_W - 1) // STRIP_W
SKEW = 4                    # chunks between z production and reduction
BS = CHF + 4                # 504: block stride in flat U/V/z/w layouts

LAST_RESULTS = None  # set by kernel() for test harness inspection


def _make_pe_np(n, d):
    pos = np.arange(n, dtype=np.float32)[:, None]
    div = np.exp(
        np.arange(0, d, 2, dtype=np.float32) * (-np.log(np.float32(10000.0)) / d)
    )
    pe = np.zeros((n, d), dtype=np.float32)
    pe[:, 0::2] = np.sin(pos * div)
    pe[:, 1::2] = np.cos(pos * div)
    return pe


def _install_profile_shim():
    """Best-effort: register the NTFF profile hook this container's antenv
    lacks, so BASS_TRACE=1 produces exec_time_ns instead of crashing."""
    try:
        import types

        if "antenv.axon_hooks" in sys.modules:
            return
        if "/root/.axon_site" not in sys.path:
            sys.path.insert(0, "/root/.axon_site")
        from trn_agent_boot.trn_boot import _ntff_profile_via_ctypes

        hook = _ntff_profile_via_ctypes("/opt/axon/libaxon_pjrt.so")
        mod = types.ModuleType("antenv.axon_hooks")
        mod.get_axon_ntff_profile_hook = lambda: hook
        mod.set_axon_ntff_profile_hook = lambda h: None
        sys.modules["antenv.axon_hooks"] = mod
        import antenv

        antenv.axon_hooks = mod
        import concourse.bass_utils as _bu

        _bu.upload_artifacts = lambda d: f"local://{d}"
    except Exception:
        pass


_install_profile_shim()

_PROG_CACHE = None


def _build_program():
    """Build the (shape-only) Bass program once; weights arrive via in_maps."""
    nc = bacc.Bacc(
        "TRN2",
        target_bir_lowering=False,
        debug=False,
        enable_asserts=False,
        num_devices=NCORES,
    )

    d_in = {}

    def din(name, shape, dt):
        d_in[name] = nc.dram_tensor(name, list(shape), dt, kind="ExternalInput").ap()
        return d_in[name]

    XH = din("XH", (128, XCOLS), BF16)
    SU_d = [din(f"SU{b}", (128, 128), BF16) for b in (0, 1)]
    SV_d = [din(f"SV{b}", (128, 128), BF16) for b in (0, 1)]
    SPQY_d = din("SPQY", (128, 8), BF16)
    CO_d = din("CO", (128, 2), BF16)
    outsT_dram = nc.dram_tensor("outsT", [3, ROWS], F32, kind="ExternalOutput").ap()
    outsP_dram = nc.dram_tensor("outsP", [5, ROWS], F32, kind="ExternalOutput").ap()

    with tile.TileContext(nc) as tc, ExitStack() as ctx:
        cpool = ctx.enter_context(tc.tile_pool(name="consts", bufs=1))
        spool = ctx.enter_context(tc.tile_pool(name="strips", bufs=1))
        up_pool = ctx.enter_context(
            tc.tile_pool(name="up", bufs=2, space=bass.MemorySpace.PSUM)
        )
        vp_pool = ctx.enter_context(
            tc.tile_pool(name="vp", bufs=1, space=bass.MemorySpace.PSUM)
        )
        tb_pool = ctx.enter_context(
            tc.tile_pool(name="tb", bufs=2, space=bass.MemorySpace.PSUM)
        )
        usb_pool = ctx.enter_context(tc.tile_pool(name="usb", bufs=3))
        vsb_pool = ctx.enter_context(tc.tile_pool(name="vsb", bufs=3))
        z_pool = ctx.enter_context(tc.tile_pool(name="z", bufs=SKEW + 1))
        w_pool = ctx.enter_context(tc.tile_pool(name="w", bufs=2))
        st_pool = ctx.enter_context(tc.tile_pool(name="st", bufs=3))

        def cload(name, dram_ap, shape, dt):
            t = cpool.tile(list(shape), dt, tag=f"c_{name}")
            nc.sync.dma_start(t[:], dram_ap[:])
            return t

        # tiny head tile first (gates chunks 0-1), then consts, then strips
        strips = [None] * NSTRIPS
        head = spool.tile([128, 2 * CHF + 6], BF16, tag="head")
        nc.sync.dma_start(head[:], XH[:, 0 : 2 * CHF + 6])

        def load_strip(s):
            a = STRIP_W * s
            w = min(a + STRIP_W + 2, XCOLS) - a
            t = spool.tile([128, w], BF16, tag=f"strip{s}")
            nc.sync.dma_start(t[:], XH[:, a : a + w])
            strips[s] = t

        SU = [cload(f"su{b}", SU_d[b], (128, 128), BF16) for b in (0, 1)]
        SV = [cload(f"sv{b}", SV_d[b], (128, 128), BF16) for b in (0, 1)]
        SPQY = cload("spqy", SPQY_d, (128, 8), BF16)
        CO = cload("co", CO_d, (128, 2), BF16)
        for s in range(NSTRIPS):
            load_strip(s)

        def moving_for(ci):
            if ci < 2:
                return head, ci * CHF
            si, k = divmod(ci, 7)
            return strips[si], k * CHF

        def front(ci, c0, F):
            st_t, o = moving_for(ci)
            up = up_pool.tile([128, 2, 512], F32, tag="up")
            vp = vp_pool.tile([128, 2, 512], F32, tag="vp")
            for b in (0, 1):
                nc.tensor.matmul(up[:, b, 0 : F + 2], SU[b][:],
                                 st_t[:, o : o + F + 2], start=True, stop=True)
            for b in (0, 1):
                nc.tensor.matmul(vp[:, b, 0:F], SV[b][:],
                                 st_t[:, o + 1 : o + F + 1], start=True, stop=True)
            # flat SBUF layouts (block stride BS) so every DVE AP coalesces
            # to one contiguous free dim -> DVE 2x/4x fast modes
            U = usb_pool.tile([128, 2 * BS + 4], BF16, tag="usb")
            V = vsb_pool.tile([128, 2 * BS + 4], BF16, tag="vsb")
            z = z_pool.tile([128, 6 * BS], BF16, tag="z")
            W2 = 2 * BS
            Uv = U[:, 0 : 2 * BS].rearrange("p (b f) -> p b f", b=2)
            Vv = V[:, 0 : 2 * BS].rearrange("p (b f) -> p b f", b=2)
            nc.scalar.copy(Uv[:, :, 0 : F + 2], up[:, :, 0 : F + 2])
            nc.scalar.copy(Vv[:, :, 0:F], vp[:, :, 0:F])
            # all three stencil adds on DVE: GpSimd streaming SBUF in
            # parallel degrades concurrent DVE ops ~4x (port contention),
            # and routing z_s through a SWDGE accum-DMA costs more in
            # pipeline latency than the 0.7us DVE op it saves
            nc.vector.tensor_tensor(z[:, 0:W2], U[:, 0:W2],
                                    V[:, 0:W2], mybir.AluOpType.add)
            nc.vector.tensor_tensor(z[:, W2 : 2 * W2], U[:, 1 : W2 + 1],
                                    V[:, 0:W2], mybir.AluOpType.add)
            nc.vector.tensor_tensor(z[:, 2 * W2 : 3 * W2], U[:, 2 : W2 + 2],
                                    V[:, 0:W2], mybir.AluOpType.add)
            return (ci, c0, F, st_t, o, z)

        def back(item):
            ci, c0, F, st_t, o, z = item
            w = w_pool.tile([128, 6 * BS], BF16, tag="w")
            I16 = mybir.dt.int16
            nc.vector.tensor_scalar(w[:, :].bitcast(I16),
                                    z[:, :].bitcast(I16), 0x7FFF, None,
                                    mybir.AluOpType.bitwise_and)
            tb = tb_pool.tile([128, 512], F32, tag="tb")
            nc.tensor.matmul(tb[96:101, 0:F], SPQY[:, 0:5],
                             st_t[:, o + 1 : o + F + 1], start=True, stop=True,
                             tile_position=(0, 96))
            W2 = 2 * BS
            for p0, sreg in ((0, 0), (32, 2 * W2), (64, W2)):
                nc.tensor.matmul(tb[p0 : p0 + 1, 0:F], CO[:, 0:1],
                                 w[:, sreg : sreg + F], start=True, stop=False)
                nc.tensor.matmul(tb[p0 : p0 + 1, 0:F], CO[:, 1:2],
                                 w[:, sreg + BS : sreg + BS + F],
                                 start=False, stop=True)
            st = st_pool.tile([128, 512], F32, tag="st")
            nc.scalar.copy(st[:, 0:F], tb[:, 0:F])
            nc.sync.dma_start(outsT_dram[0:3, c0 : c0 + F], st[0:96:32, 0:F])
            nc.sync.dma_start(outsP_dram[:, c0 : c0 + F], st[96:101, 0:F])

        pend = deque()
        for ci, (c0, F) in enumerate(CHUNKS):
            pend.append(front(ci, c0, F))
            if len(pend) > SKEW:
                back(pend.popleft())
        while pend:
            back(pend.popleft())

    nc.compile()
    return nc


def _get_program():
    global _PROG_CACHE
    if _PROG_CACHE is None:
        _PROG_CACHE = _build_program()
    return _PROG_CACHE


def kernel(x, W_exp, b_exp, W_l, b_l, W_r, b_r, att, bias, W_fc, b_fc):
    global LAST_RESULTS
    x = np.asarray(x, dtype=np.float32)
    W_exp = np.asarray(W_exp, np.float32)
    b_exp = np.asarray(b_exp, np.float32)
    W_l = np.asarray(W_l, np.float32)
    b_l = np.asarray(b_l, np.float32)
    W_r = np.asarray(W_r, np.float32)
    b_r = np.asarray(b_r, np.float32)
    att = np.asarray(att, np.float32)
    bias = np.asarray(bias, np.float32)
    W_fc = np.asarray(W_fc, np.float32)
    b_fc = np.asarray(b_fc, np.float32)

    lw = L - 1  # only the last conv layer matters
    pe = _make_pe_np(N, H)
    a = att[lw]
    s = np.where(a >= 0.0, 1.0, -1.0).astype(np.float32)
    ahat = np.abs(a)

    Wl_full = W_exp @ W_l[lw]                     # [64,256]
    Wr_full = W_exp @ W_r[lw]
    cl = (b_exp + pe) @ W_l[lw] + b_l[lw]         # [100,256]
    cr = (b_exp + pe) @ W_r[lw] + b_r[lw]

    Wtl = Wl_full * ahat[None, :]                 # ahat-folded
    Wtr = Wr_full * ahat[None, :]
    ctl = cl * ahat[None, :]
    ctr = cr * ahat[None, :]

    wp = Wl_full @ a                              # [64]
    wq = Wr_full @ a
    Wy = Wl_full @ W_fc                           # [64,3]
    cp = cl @ a                                   # [100]
    cq = cr @ a
    cy = cl @ W_fc                                # [100,3]

    # shared rank-64 basis over n for ALL per-node biases
    T = np.concatenate(
        [ctl, ctr, cp[:, None], cq[:, None], cy], axis=1
    )                                             # [100, 517]
    U_, S_, Vt_ = np.linalg.svd(T.astype(np.float64), full_matrices=False)
    Bq = U_[:, :64].T                             # [64, 100]
    A = (Bq @ T.astype(np.float64)).astype(np.float32)
    Bq = Bq.astype(np.float32)
    A_cl = A[:, 0:256]
    A_cr = A[:, 256:512]
    A_pqy = A[:, 512:517]                         # cp, cq, cy coeffs

    def bf(arr):
        return np.ascontiguousarray(arr.astype(NPBF16))

    consts = {}
    for b in (0, 1):
        sl = slice(b * 128, (b + 1) * 128)
        consts[f"SU{b}"] = bf(np.concatenate([Wtl[:, sl], A_cl[:, sl]], axis=0))
        consts[f"SV{b}"] = bf(np.concatenate([Wtr[:, sl], A_cr[:, sl]], axis=0))
    SPQY = np.zeros((128, 8), np.float32)
    SPQY[0:64, 0] = wp
    SPQY[0:64, 1] = wq
    SPQY[0:64, 2:5] = Wy
    SPQY[64:128, 0:5] = A_pqy
    consts["SPQY"] = bf(SPQY)
    CO = np.zeros((128, 2), np.float32)
    CO[:, 0] = s[0:128]
    CO[:, 1] = s[128:256]
    consts["CO"] = bf(CO)

    # per-core XH: [128, 1+ROWS+1]; rows 0:64 x^T (shifted +1 col, zero
    # guards), rows 64:128 the n-periodic basis aligned to the same cols
    xr = x.reshape(NCORES, ROWS, IN)
    n_pat = np.arange(XCOLS) % N                  # phase of col c is (c-1)%N
    basis_cols = bf(Bq[:, (n_pat - 1) % N])       # [64, XCOLS]
    in_maps = []
    for c in range(NCORES):
        XHc = np.zeros((128, XCOLS), NPBF16)
        XHc[0:64, 1 : 1 + ROWS] = bf(xr[c].T)
        XHc[64:128, :] = basis_cols
        XHc[64:128, 0] = 0
        XHc[64:128, XCOLS - 1] = 0
        m = dict(consts)
        m["XH"] = np.ascontiguousarray(XHc)
        in_maps.append(m)

    nc = _get_program()
    res = None
    last_exc = None
    for attempt in range(3):
        try:
            res = run_bass_kernel_spmd(
                nc,
                in_maps,
                core_ids=list(range(NCORES)),
            )
            break
        except Exception as e:  # transient device-unrecoverable on first NEFF run
            last_exc = e
            import time as _time

            _time.sleep(2.0)
    if res is None:
        raise last_exc
    LAST_RESULTS = res

    # ---------------- host tail ----------------
    n_of_r = np.tile(np.arange(N), BC)                        # [ROWS]

    out_all = np.empty((B, C), np.float32)
    for c in range(NCORES):
        oT = np.asarray(res.results[c]["outsT"], np.float32)  # [3, ROWS]
        oP = np.asarray(res.results[c]["outsP"], np.float32)  # [5, ROWS]
        t_l, t_r, t_s = oT[0], oT[1], oT[2]
        Pb, Qb = oP[0], oP[1]                                 # biases included
        Y = oP[2:5].T                                         # [ROWS,3]

        Pb_m1 = np.roll(Pb, 1)                                # P at source row r-1
        Pb_p1 = np.roll(Pb, -1)

        lg_l = 0.6 * (Pb_m1 + Qb) + 0.4 * t_l
        lg_r = 0.6 * (Pb_p1 + Qb) + 0.4 * t_r
        lg_s = 0.6 * (Pb + Qb) + 0.4 * t_s

        lg_l = np.where(n_of_r == 0, -np.inf, lg_l)
        lg_r = np.where(n_of_r == N - 1, -np.inf, lg_r)

        mx = np.maximum(np.maximum(lg_l, lg_r), lg_s)
        el = np.exp(lg_l - mx)
        er = np.exp(lg_r - mx)
        es = np.exp(lg_s - mx)
        den = el + er + es
        al, ar, asf = el / den, er / den, es / den

        Y_m1 = np.roll(Y, 1, axis=0)
        Y_p1 = np.roll(Y, -1, axis=0)
        msgs = al[:, None] * Y_m1 + ar[:, None] * Y_p1 + asf[:, None] * Y
        pooled = msgs.reshape(BC, N, C).sum(axis=1)
        out_all[c * BC : (c + 1) * BC] = (
            pooled + N * (bias[lw] @ W_fc)[None, :] + b_fc[None, :]
        )
    return out_all


# revision 32
# speedup vs baseline: 1.0573x; 1.0022x over previous
"""Trainium2 Bass kernel for nn_GATModel (GATv2 on a bidirectional chain graph).

Key algebraic facts exploited (derived from the reference):
  * The reference's conv loop feeds x0 into EVERY layer, so only the LAST
    GATv2 layer (index L-1) affects the output.
  * x0 = x @ W_exp + b_exp + pe never needs materializing:
        xl = x0 @ Wl + bl = x @ (W_exp@Wl) + [(b_exp+pe[n])@Wl + bl]
  * The graph is a chain + self loops, so message passing is a 3-tap stencil
    (left / self / right) with a masked 3-way softmax per node.
  * a . leaky_relu(z) = 0.6*(a . z) + 0.4*(a . |z|)   (slope 0.2)
    and with ahat=|a| folded into the weight columns, a_h*|z_h| =
    sign(a_h)*|ztilde_h|.
  * Every per-node bias (cl~, cr~, cp, cq, cy) is a fixed function of
    n built from pe rows + constants, so they all live in one shared
    rank<=64 basis Bq over n. Stacking Bq[:, n] under x[j] in the moving
    tile lets ONE K=128 matmul produce x@W + bias(n) exactly.

Device pipeline per 500-row chunk (col-major: [h-part, row-free]):
  PE: u_b = [x;Bq] @ [Wtl_b; A_cl_b]  (2 matmuls, bias included, +2 halo
      cols), v_b likewise with Wtr/A_cr  -> PSUM f32
  ACT: evacuate u, v -> SBUF bf16 (flat layouts, block stride BS so all
      DVE access patterns coalesce to one contiguous free dim)
  DVE: 3 stencil adds z_{l,s,r} = shift(u)+v (bf16 2x mode, ~0.52ns/elem)
      + one mega-abs over all 3 regions (int16-view AND 0x7FFF, 4x mode)
  PE: t_sigma = sum_h sign(a_h)|z| via M=1 matmuls with bf16 moving
      (1 cyc/col vs 4 for f32) + P,Q,Y matmul (bias folded via Bq rows)
  ACT: evac tb -> DMA out.  GpSimd stays IDLE: its SBUF streaming
      degrades concurrent DVE fast-mode ops ~4x (port contention).
The loop is software-pipelined with SKEW chunks between the z adds and
the reductions; engines run ~80% busy with DVE/ACT co-limiting.
Host finishes: logits = 0.6(p+q) + 0.4 t, masks, 3-way softmax, alpha-
weighted message pooling, final fc - O(B*N) work; all O(B*N*H) is on HW.

Note: the first execution of a freshly compiled NEFF intermittently hits
NRT_EXEC_UNIT_UNRECOVERABLE on this axon stack; kernel() retries.
"""

import os
import sys

sys.path.insert(0, "/opt/trn_rl_repo")

from collections import deque  # noqa: E402
from contextlib import ExitStack  # noqa: E402

import ml_dtypes  # noqa: E402
import numpy as np  # noqa: E402

import concourse.bass as bass  # noqa: E402
import concourse.tile as tile  # noqa: E402
from concourse import bacc, mybir  # noqa: E402
from concourse.bass_utils import run_bass_kernel_spmd  # noqa: E402

BF16 = mybir.dt.bfloat16
F32 = mybir.dt.float32
NPBF16 = ml_dtypes.bfloat16

B, N, IN, H, L, C = 2048, 100, 64, 256, 3, 3
NEG = 0.2
NCORES = 8
BC = B // NCORES            # 256 graphs per core
ROWS = BC * N               # 25600 rows per core
CH_ELEMS = 5
CHF = CH_ELEMS * N          # 500 rows per chunk
NFULL = BC // CH_ELEMS      # 51 full chunks
REM_ELEMS = BC - NFULL * CH_ELEMS   # 1 leftover graph
CHUNKS = [(i * CHF, CHF) for i in range(NFULL)]
if REM_ELEMS:
    CHUNKS.append((NFULL * CHF, REM_ELEMS * N))

XCOLS = 1 + ROWS + 1        # zero guard columns at 0 and ROWS+1
STRIP_W = 7 * CHF           # 3500: strips aligned to 7 chunks
NSTRIPS = (ROWS + STRIP_W - 1) // STRIP_W
SKEW = 4                    # chunks between z production and reduction
BS = CHF + 4                # 504: block stride in flat U/V/z/w layouts

LAST_RESULTS = None  # set by kernel() for test harness inspection


def _make_pe_np(n, d):
    pos = np.arange(n, dtype=np.float32)[:, None]
    div = np.exp(
        np.arange(0, d, 2, dtype=np.float32) * (-np.log(np.float32(10000.0)) / d)
    )
    pe = np.zeros((n, d), dtype=np.float32)
    pe[:, 0::2] = np.sin(pos * div)
    pe[:, 1::2] = np.cos(pos * div)
    return pe


def _install_profile_shim():
    """Best-effort: register the NTFF profile hook this container's antenv
    lacks, so BASS_TRACE=1 produces exec_time_ns instead of crashing."""
    try:
        import types

        if "antenv.axon_hooks" in sys.modules:
            return
        if "/root/.axon_site" not in sys.path:
            sys.path.insert(0, "/root/.axon_site")
        from trn_agent_boot.trn_boot import _ntff_profile_via_ctypes

        hook = _ntff_profile_via_ctypes("/opt/axon/libaxon_pjrt.so")
        mod = types.ModuleType("antenv.axon_hooks")
        mod.get_axon_ntff_profile_hook = lambda: hook
        mod.set_axon_ntff_profile_hook = lambda h: None
        sys.modules["antenv.axon_hooks"] = mod
        import antenv

        antenv.axon_hooks = mod
        import concourse.bass_utils as _bu

        _bu.upload_artifacts = lambda d: f"local://{d}"
    except Exception:
        pass


_install_profile_shim()

_PROG_CACHE = None


def _build_program():
    """Build the (shape-only) Bass program once; weights arrive via in_maps."""
    nc = bacc.Bacc(
        "TRN2",
        target_bir_lowering=False,
        debug=False,
        enable_asserts=False,
        num_devices=NCORES,
    )

    d_in = {}

    def din(name, shape, dt):
        d_in[name] = nc.dram_tensor(name, list(shape), dt, kind="ExternalInput").ap()
        return d_in[name]

    XH = din("XH", (128, XCOLS), BF16)
    SU_d = [din(f"SU{b}", (128, 128), BF16) for b in (0, 1)]
    SV_d = [din(f"SV{b}", (128, 128), BF16) for b in (0, 1)]
    SPQY_d = din("SPQY", (128, 8), BF16)
    CO_d = din("CO", (128, 2), BF16)
    outsT_dram = nc.dram_tensor("outsT", [3, ROWS], F32, kind="ExternalOutput").ap()
    outsP_dram = nc.dram_tensor("outsP", [5, ROWS], F32, kind="ExternalOutput").ap()

    with tile.TileContext(nc) as tc, ExitStack() as ctx:
        cpool = ctx.enter_context(tc.tile_pool(name="consts", bufs=1))
        spool = ctx.enter_context(tc.tile_pool(name="strips", bufs=1))
        up_pool = ctx.enter_context(
            tc.tile_pool(name="up", bufs=2, space=bass.MemorySpace.PSUM)
        )
        vp_pool = ctx.enter_context(
            tc.tile_pool(name="vp", bufs=1, space=bass.MemorySpace.PSUM)
        )
        tb_pool = ctx.enter_context(
            tc.tile_pool(name="tb", bufs=2, space=bass.MemorySpace.PSUM)
        )
        usb_pool = ctx.enter_context(tc.tile_pool(name="usb", bufs=3))
        vsb_pool = ctx.enter_context(tc.tile_pool(name="vsb", bufs=3))
        z_pool = ctx.enter_context(tc.tile_pool(name="z", bufs=SKEW + 1))
        w_pool = ctx.enter_context(tc.tile_pool(name="w", bufs=2))
        st_pool = ctx.enter_context(tc.tile_pool(name="st", bufs=3))

        def cload(name, dram_ap, shape, dt):
            t = cpool.tile(list(shape), dt, tag=f"c_{name}")
            nc.sync.dma_start(t[:], dram_ap[:])
            return t

        # tiny head tile first (gates chunks 0-1), then consts, then strips
        strips = [None] * NSTRIPS
        head = spool.tile([128, 2 * CHF + 6], BF16, tag="head")
        nc.sync.dma_start(head[:], XH[:, 0 : 2 * CHF + 6])

        def load_strip(s):
            a = STRIP_W * s
            w = min(a + STRIP_W + 2, XCOLS) - a
            t = spool.tile([128, w], BF16, tag=f"strip{s}")
            nc.sync.dma_start(t[:], XH[:, a : a + w])
            strips[s] = t

        SU = [cload(f"su{b}", SU_d[b], (128, 128), BF16) for b in (0, 1)]
        SV = [cload(f"sv{b}", SV_d[b], (128, 128), BF16) for b in (0, 1)]
        SPQY = cload("spqy", SPQY_d, (128, 8), BF16)
        CO = cload("co", CO_d, (128, 2), BF16)
        for s in range(NSTRIPS):
            load_strip(s)

        def moving_for(ci):
            if ci < 2:
                return head, ci * CHF
            si, k = divmod(ci, 7)
            return strips[si], k * CHF

        def front(ci, c0, F):
            st_t, o = moving_for(ci)
            up = up_pool.tile([128, 2, 512], F32, tag="up")
            vp = vp_pool.tile([128, 2, 512], F32, tag="vp")
            for b in (0, 1):
                nc.tensor.matmul(up[:, b, 0 : F + 2], SU[b][:],
                                 st_t[:, o : o + F + 2], start=True, stop=True)
            for b in (0, 1):
                nc.tensor.matmul(vp[:, b, 0:F], SV[b][:],
                                 st_t[:, o + 1 : o + F + 1], start=True, stop=True)
            # flat SBUF layouts (block stride BS) so every DVE AP coalesces
            # to one contiguous free dim -> DVE 2x/4x fast modes
            U = usb_pool.tile([128, 2 * BS + 4], BF16, tag="usb")
            V = vsb_pool.tile([128, 2 * BS + 4], BF16, tag="vsb")
            z = z_pool.tile([128, 6 * BS], BF16, tag="z")
            W2 = 2 * BS
            Uv = U[:, 0 : 2 * BS].rearrange("p (b f) -> p b f", b=2)
            Vv = V[:, 0 : 2 * BS].rearrange("p (b f) -> p b f", b=2)
            nc.scalar.copy(Uv[:, :, 0 : F + 2], up[:, :, 0 : F + 2])
            nc.scalar.copy(Vv[:, :, 0:F], vp[:, :, 0:F])
            # all three stencil adds on DVE: GpSimd streaming SBUF in
            # parallel degrades concurrent DVE ops ~4x (port contention),
            # and routing z_s through a SWDGE accum-DMA costs more in
            # pipeline latency than the 0.7us DVE op it saves
            nc.vector.tensor_tensor(z[:, 0:W2], U[:, 0:W2],
                                    V[:, 0:W2], mybir.AluOpType.add)
            nc.vector.tensor_tensor(z[:, W2 : 2 * W2], U[:, 1 : W2 + 1],
                                    V[:, 0:W2], mybir.AluOpType.add)
            nc.vector.tensor_tensor(z[:, 2 * W2 : 3 * W2], U[:, 2 : W2 + 2],
                                    V[:, 0:W2], mybir.AluOpType.add)
            return (ci, c0, F, st_t, o, z)

        def back(item):
            ci, c0, F, st_t, o, z = item
            w = w_pool.tile([128, 6 * BS], BF16, tag="w")
            I16 = mybir.dt.int16
            nc.vector.tensor_scalar(w[:, :].bitcast(I16),
                                    z[:, :].bitcast(I16), 0x7FFF, None,
                                    mybir.AluOpType.bitwise_and)
            tb = tb_pool.tile([128, 512], F32, tag="tb")
            nc.tensor.matmul(tb[96:101, 0:F], SPQY[:, 0:5],
                             st_t[:, o + 1 : o + F + 1], start=True, stop=True,
                             tile_position=(0, 96))
            W2 = 2 * BS
            for p0, sreg in ((0, 0), (32, 2 * W2), (64, W2)):
                nc.tensor.matmul(tb[p0 : p0 + 1, 0:F], CO[:, 0:1],
                                 w[:, sreg : sreg + F], start=True, stop=False)
                nc.tensor.matmul(tb[p0 : p0 + 1, 0:F], CO[:, 1:2],
                                 w[:, sreg + BS : sreg + BS + F],
                                 start=False, stop=True)
            st = st_pool.tile([128, 512], F32, tag="st")
            nc.scalar.copy(st[:, 0:F], tb[:, 0:F])
            nc.sync.dma_start(outsT_dram[0:3, c0 : c0 + F], st[0:96:32, 0:F])
            nc.sync.dma_start(outsP_dram[:, c0 : c0 + F], st[96:101, 0:F])

        pend = deque()
        for ci, (c0, F) in enumerate(CHUNKS):
            pend.append(front(ci, c0, F))
            if len(pend) > SKEW:
                back(pend.popleft())
        while pend:
            back(pend.popleft())

    nc.compile()
    return nc


def _get_program():
    global _PROG_CACHE
    if _PROG_CACHE is None:
        _PROG_CACHE = _build_program()
    return _PROG_CACHE


def kernel(x, W_exp, b_exp, W_l, b_l, W_r, b_r, att, bias, W_fc, b_fc):
    global LAST_RESULTS
    x = np.asarray(x, dtype=np.float32)
    W_exp = np.asarray(W_exp, np.float32)
    b_exp = np.asarray(b_exp, np.float32)
    W_l = np.asarray(W_l, np.float32)
    b_l = np.asarray(b_l, np.float32)
    W_r = np.asarray(W_r, np.float32)
    b_r = np.asarray(b_r, np.float32)
    att = np.asarray(att, np.float32)
    bias = np.asarray(bias, np.float32)
    W_fc = np.asarray(W_fc, np.float32)
    b_fc = np.asarray(b_fc, np.float32)

    lw = L - 1  # only the last conv layer matters
    pe = _make_pe_np(N, H)
    a = att[lw]
    s = np.where(a >= 0.0, 1.0, -1.0).astype(np.float32)
    ahat = np.abs(a)

    Wl_full = W_exp @ W_l[lw]                     # [64,256]
    Wr_full = W_exp @ W_r[lw]
    cl = (b_exp + pe) @ W_l[lw] + b_l[lw]         # [100,256]
    cr = (b_exp + pe) @ W_r[lw] + b_r[lw]

    Wtl = Wl_full * ahat[None, :]                 # ahat-folded
    Wtr = Wr_full * ahat[None, :]
    ctl = cl * ahat[None, :]
    ctr = cr * ahat[None, :]

    wp = Wl_full @ a                              # [64]
    wq = Wr_full @ a
    Wy = Wl_full @ W_fc                           # [64,3]
    cp = cl @ a                                   # [100]
    cq = cr @ a
    cy = cl @ W_fc                                # [100,3]

    # shared rank-64 basis over n for ALL per-node biases
    T = np.concatenate(
        [ctl, ctr, cp[:, None], cq[:, None], cy], axis=1
    )                                             # [100, 517]
    U_, S_, Vt_ = np.linalg.svd(T.astype(np.float64), full_matrices=False)
    Bq = U_[:, :64].T                             # [64, 100]
    A = (Bq @ T.astype(np.float64)).astype(np.float32)
    Bq = Bq.astype(np.float32)
    A_cl = A[:, 0:256]
    A_cr = A[:, 256:512]
    A_pqy = A[:, 512:517]                         # cp, cq, cy coeffs

    def bf(arr):
        return np.ascontiguousarray(arr.astype(NPBF16))

    consts = {}
    for b in (0, 1):
        sl = slice(b * 128, (b + 1) * 128)
        consts[f"SU{b}"] = bf(np.concatenate([Wtl[:, sl], A_cl[:, sl]], axis=0))
        consts[f"SV{b}"] = bf(np.concatenate([Wtr[:, sl], A_cr[:, sl]], axis=0))
    SPQY = np.zeros((128, 8), np.float32)
    SPQY[0:64, 0] = wp
    SPQY[0:64, 1] = wq
    SPQY[0:64, 2:5] = Wy
    SPQY[64:128, 0:5] = A_pqy
    consts["SPQY"] = bf(SPQY)
    CO = np.zeros((128, 2), np.float32)
    CO[:, 0] = s[0:128]
    CO[:, 1] = s[128:256]
    consts["CO"] = bf(CO)

    # per-core XH: [128, 1+ROWS+1]; rows 0:64 x^T (shifted +1 col, zero
    # guards), rows 64:128 the n-periodic basis aligned to the same cols
    xr = x.reshape(NCORES, ROWS, IN)
    n_pat = np.arange(XCOLS) % N                  # phase of col c is (c-1)%N
    basis_cols = bf(Bq[:, (n_pat - 1) % N])       # [64, XCOLS]
    in_maps = []
    for c in range(NCORES):
        XHc = np.zeros((128, XCOLS), NPBF16)
        XHc[0:64, 1 : 1 + ROWS] = bf(xr[c].T)
        XHc[64:128, :] = basis_cols
        XHc[64:128, 0] = 0
        XHc[64:128, XCOLS - 1] = 0
        m = dict(consts)
        m["XH"] = np.ascontiguousarray(XHc)
        in_maps.append(m)

    nc = _get_program()
    res = None
    last_exc = None
    for attempt in range(3):
        try:
            res = run_bass_kernel_spmd(
                nc,
                in_maps,
                core_ids=list(range(NCORES)),
            )
            break
        except Exception as e:  # transient device-unrecoverable on first NEFF run
            last_exc = e
            import time as _time

            _time.sleep(2.0)
    if res is None:
        raise last_exc
    LAST_RESULTS = res

    # ---------------- host tail ----------------
    n_of_r = np.tile(np.arange(N), BC)                        # [ROWS]

    out_all = np.empty((B, C), np.float32)
    for c in range(NCORES):
        oT = np.asarray(res.results[c]["outsT"], np.float32)  # [3, ROWS]
        oP = np.asarray(res.results[c]["outsP"], np.float32)  # [5, ROWS]
        t_l, t_r, t_s = oT[0], oT[1], oT[2]
        Pb, Qb = oP[0], oP[1]                                 # biases included
        Y = oP[2:5].T                                         # [ROWS,3]

        Pb_m1 = np.roll(Pb, 1)                                # P at source row r-1
        Pb_p1 = np.roll(Pb, -1)

        lg_l = 0.6 * (Pb_m1 + Qb) + 0.4 * t_l
        lg_r = 0.6 * (Pb_p1 + Qb) + 0.4 * t_r
        lg_s = 0.6 * (Pb + Qb) + 0.4 * t_s

        lg_l = np.where(n_of_r == 0, -np.inf, lg_l)
        lg_r = np.where(n_of_r == N - 1, -np.inf, lg_r)

        mx = np.maximum(np.maximum(lg_l, lg_r), lg_s)
        el = np.exp(lg_l - mx)
        er = np.exp(lg_r - mx)
        es = np.exp(lg_s - mx)
        den = el + er + es
        al, ar, asf = el / den, er / den, es / den

        Y_m1 = np.roll(Y, 1, axis=0)
        Y_p1 = np.roll(Y, -1, axis=0)
        msgs = al[:, None] * Y_m1 + ar[:, None] * Y_p1 + asf[:, None] * Y
        pooled = msgs.reshape(BC, N, C).sum(axis=1)
        out_all[c * BC : (c + 1) * BC] = (
            pooled + N * (bias[lw] @ W_fc)[None, :] + b_fc[None, :]
        )
    return out_all
